# revision 8
# baseline (speedup 1.0000x reference)
"""CopyGenerator on 8 TRN2 NeuronCores.

Strategy: tensor-parallel split of the 50257-wide generator vocab across the
8 cores (6400 padded columns each).  Each core:
  - holds its W_gen shard resident in SBUF as fp8 (pre-scaled x32 on host,
    pre-transposed to [128, DT, VS]),
  - computes logits = hidden @ W_shard.T with fp8 DoubleRow matmuls
    (K=256 per instruction, fp32 PSUM accum, 4-bank psum tiles),
  - applies exp on the Scalar engine (scale=1/32 undoes the W prescale),
    writing RAW (unnormalised) exp values as bf16 straight to DRAM,
  - computes the copy-gate logits z = hidden @ W_copy.T in bf16 (accuracy:
    the copy path dominates the output magnitude) and the copy-attention
    bmm attn^T @ src_map in bf16, both written out raw.
The softmax denominator (a cross-shard sum) and the per-row (1-p_copy)/denom
and p_copy scalings are applied on the host while gathering/unsharding the
8 per-core outputs into the full [2048, 50321] float32 output.  A nonzero
b_gen factorizes exactly as a per-column exp(b_gen) scale, also host-side.
PAD column handling: its W row is zeroed on the host => exp 1, and the host
zeroes the column and excludes it from the denominator.
"""

import os
import sys

for _p in ("/opt/trn_rl_repo", "/opt/trn_rl_repo/concourse"):
    if _p not in sys.path:
        sys.path.insert(0, _p)

from contextlib import ExitStack

import ml_dtypes
import numpy as np

import concourse.bass as bass
import concourse.mybir as mybir
import concourse.tile as tile
from concourse import bacc
from concourse.bass_utils import run_bass_kernel_spmd

# ---- problem constants (hardcoded per the self-contained-kernel contract) ----
N, D = 2048, 1024                 # tlen*batch rows, hidden dim
TLEN, BATCH, SLEN, CVOCAB = 64, 32, 128, 64
VOCAB = 50257
PAD_IDX = 0
NCORES = 8
VS = 6400                         # per-core padded vocab shard width
VPAD = VS * NCORES                # 51200
DT = D // 128                     # 8 contraction tiles
NT = N // 128                     # 16 row tiles
WSCALE = 32.0                     # host premultiplies W_gen; exp scale undoes

CHUNK = 2048                      # psum tile width (4 banks)
TAIL = VS - 3 * CHUNK             # 256

BF16 = ml_dtypes.bfloat16
FP8 = ml_dtypes.float8_e4m3
F32 = mybir.dt.float32
BF16_T = mybir.dt.bfloat16
FP8_T = mybir.dt.float8e4

LAST_RESULTS = None               # BassKernelResults of the most recent run
_NC_CACHE = {}


def _build():
    nc = bacc.Bacc("TRN2", target_bir_lowering=False, debug=False,
                   num_devices=NCORES)

    wt = nc.dram_tensor("wt", [128, DT, VS], FP8_T, kind="ExternalInput").ap()
    ht8 = nc.dram_tensor("ht8", [128, DT, N], FP8_T, kind="ExternalInput").ap()
    ht16 = nc.dram_tensor("ht16", [128, DT, N], BF16_T,
                          kind="ExternalInput").ap()
    attn_r = nc.dram_tensor("attn_r", [128, BATCH * TLEN], BF16_T,
                            kind="ExternalInput").ap()
    smap = nc.dram_tensor("smap", [128, BATCH * CVOCAB], BF16_T,
                          kind="ExternalInput").ap()
    wc = nc.dram_tensor("wc", [128, DT], BF16_T, kind="ExternalInput").ap()
    # transposed layout [vocab_shard, rows]: the host untransposes.  This
    # makes wt the STATIONARY matmul operand so one PE weight load serves
    # 4 moving matmuls (LDWEIGHTS serialize with matmuls on hw).
    out_exp = nc.dram_tensor("out_exp", [VS, N], BF16_T,
                             kind="ExternalOutput").ap()
    zout = nc.dram_tensor("zout", [1, N], F32, kind="ExternalOutput").ap()
    cpout = nc.dram_tensor("cpout", [N, CVOCAB], F32,
                           kind="ExternalOutput").ap()

    with tile.TileContext(nc) as tc, ExitStack() as ctx:
        singles = ctx.enter_context(tc.tile_pool(name="singles", bufs=1))

        # ---- resident inputs.  Order matters: the first matmuls need ht8
        # (all d, first 128 n-cols) and wt chunk 0, so load those first. ----
        ht8_sb = singles.tile([128, DT, N], FP8_T)
        nc.sync.dma_start(out=ht8_sb[:, :, 0:128], in_=ht8[:, :, 0:128])
        wt_sb = singles.tile([128, DT, VS], FP8_T)
        nc.sync.dma_start(out=wt_sb[:, :, 0:CHUNK], in_=wt[:, :, 0:CHUNK])
        nc.sync.dma_start(out=ht8_sb[:, :, 128:N], in_=ht8[:, :, 128:N])
        for c0 in range(CHUNK, VS, CHUNK):
            cw = min(CHUNK, VS - c0)
            nc.sync.dma_start(out=wt_sb[:, :, c0:c0 + cw],
                              in_=wt[:, :, c0:c0 + cw])
        ht16_sb = singles.tile([128, DT, N], BF16_T)
        nc.sync.dma_start(out=ht16_sb, in_=ht16)
        attn_sb = singles.tile([128, BATCH * TLEN], BF16_T)
        nc.sync.dma_start(out=attn_sb, in_=attn_r)
        sm_sb = singles.tile([128, BATCH * CVOCAB], BF16_T)
        nc.sync.dma_start(out=sm_sb, in_=smap)
        wc_sb = singles.tile([128, DT], BF16_T)
        nc.sync.dma_start(out=wc_sb, in_=wc)

        z_sb = singles.tile([1, N], F32)
        cp_sb = singles.tile([TLEN, BATCH * CVOCAB], F32)

        expp = ctx.enter_context(tc.tile_pool(name="expp", bufs=3))
        ps = ctx.enter_context(tc.tile_pool(name="ps", bufs=2, space="PSUM"))

        def main_tile(ct):
            # one 128-wide vocab column tile x all 2048 rows
            c0 = ct * 128
            exp_t = expp.tile([128, N], BF16_T, tag="exp")
            psm = ps.tile([128, N], F32, tag="psm")
            # kpair-outer so 4 consecutive matmuls share the stationary tile;
            # only the first self-loads the PE array (ldweights=False skips
            # the redundant reload on the rest)
            for i in range(DT // 2):
                for q in range(N // 512):
                    mm = nc.tensor.matmul(
                        psm[:, q * 512:(q + 1) * 512],
                        lhsT=wt_sb[:, 2 * i:2 * i + 2, c0:c0 + 128],
                        rhs=ht8_sb[:, 2 * i:2 * i + 2,
                                   q * 512:(q + 1) * 512],
                        start=(i == 0), stop=(i == DT // 2 - 1),
                        perf_mode=mybir.MatmulPerfMode.DoubleRow,
                    )
                    if q > 0:
                        mm.ins.ldweights = False
            nc.scalar.activation(exp_t, psm,
                                 mybir.ActivationFunctionType.Exp,
                                 scale=1.0 / WSCALE)
            nc.sync.dma_start(out=out_exp[c0:c0 + 128, :], in_=exp_t)

        def z_path():
            # z = hidden @ W_copy.T in bf16, psum row 0 of a borrowed buf
            zp = ps.tile([128, CHUNK], F32, tag="psm")
            for q in range(N // 512):
                for d in range(DT):
                    nc.tensor.matmul(
                        zp[0:1, q * 512:(q + 1) * 512],
                        lhsT=wc_sb[:, d:d + 1],
                        rhs=ht16_sb[:, d, q * 512:(q + 1) * 512],
                        start=(d == 0), stop=(d == DT - 1),
                    )
            nc.vector.tensor_copy(out=z_sb, in_=zp[0:1, :])
            nc.sync.dma_start(out=zout, in_=z_sb)

        def copy_path():
            # per-batch [64t,128s] @ [128s,64c], raw (p_copy applied on host)
            cp = ps.tile([128, CHUNK], F32, tag="psm")
            for b in range(BATCH):
                nc.tensor.matmul(
                    cp[0:TLEN, b * CVOCAB:(b + 1) * CVOCAB],
                    lhsT=attn_sb[:, b * TLEN:(b + 1) * TLEN],
                    rhs=sm_sb[:, b * CVOCAB:(b + 1) * CVOCAB],
                    start=True, stop=True,
                )
            nc.vector.tensor_copy(out=cp_sb, in_=cp[0:TLEN, :])
            # cpout[(t*BATCH+b), c] = cp_sb[t, b*CVOCAB+c]
            nc.sync.dma_start(
                out=cpout.rearrange("(t b) c -> t (b c)", b=BATCH),
                in_=cp_sb)

        main_tile(0)
        z_path()
        copy_path()
        for ct in range(1, VS // 128):
            main_tile(ct)

    nc.compile()
    return nc


def _get_nc():
    if "nc" not in _NC_CACHE:
        _NC_CACHE["nc"] = _build()
    return _NC_CACHE["nc"]


def kernel(hidden, attn, src_map, W_gen, b_gen, W_copy, b_copy):
    global LAST_RESULTS
    hidden = np.asarray(hidden, dtype=np.float32)
    attn = np.asarray(attn, dtype=np.float32)
    src_map = np.asarray(src_map, dtype=np.float32)
    W_gen = np.asarray(W_gen, dtype=np.float32)
    b_gen = np.asarray(b_gen, dtype=np.float32)
    W_copy = np.asarray(W_copy, dtype=np.float32)
    b_copy = np.asarray(b_copy, dtype=np.float32)

    nc = _get_nc()

    # hidden^T, tiled: ht[p, d, n] = hidden[n, d*128 + p]
    ht = np.ascontiguousarray(hidden.reshape(N, DT, 128).transpose(2, 1, 0))
    ht8 = ht.astype(FP8)
    ht16 = ht.astype(BF16)

    # padded W (x WSCALE) with masked rows zeroed (PAD row + vocab padding)
    Wp = np.zeros((VPAD, D), dtype=np.float32)
    Wp[:VOCAB] = W_gen * WSCALE
    Wp[PAD_IDX] = 0.0

    # attn rearranged to [s, b, t]
    attn_r = np.ascontiguousarray(
        attn.reshape(TLEN, BATCH, SLEN).transpose(2, 1, 0)
    ).reshape(128, BATCH * TLEN).astype(BF16)
    smap = np.ascontiguousarray(
        src_map.reshape(SLEN, BATCH * CVOCAB)).astype(BF16)
    wc = np.ascontiguousarray(W_copy[0].reshape(DT, 128).T).astype(BF16)

    in_maps = []
    for c in range(NCORES):
        shard = Wp[c * VS:(c + 1) * VS]                      # [VS, D]
        wt_c = np.ascontiguousarray(
            shard.reshape(VS, DT, 128).transpose(2, 1, 0)).astype(FP8)
        in_maps.append({
            "wt": wt_c,
            "ht8": ht8,
            "ht16": ht16,
            "attn_r": attn_r,
            "smap": smap,
            "wc": wc,
        })

    res = run_bass_kernel_spmd(nc, in_maps, core_ids=list(range(NCORES)))
    LAST_RESULTS = res

    # ---- host-side gather/unshard + softmax finalization ----
    gen = np.empty((N, VOCAB), dtype=np.float32)
    for c in range(NCORES):
        lo = c * VS
        hi = min(lo + VS, VOCAB)
        if hi > lo:
            gen[:, lo:hi] = res.results[c]["out_exp"][:hi - lo, :].T
    gen[:, PAD_IDX] = 0.0
    if np.any(b_gen):
        bg = b_gen.astype(np.float64).copy()
        bg[PAD_IDX] = 0.0
        gen *= np.exp(bg)[None, :].astype(np.float32)
    denom = gen.sum(axis=1, dtype=np.float64)                # [N]

    z = res.results[0]["zout"][0].astype(np.float64)         # [N]
    pc = 1.0 / (1.0 + np.exp(-(z + float(b_copy.reshape(-1)[0]))))

    out = np.empty((N, VOCAB + CVOCAB), dtype=np.float32)
    out[:, :VOCAB] = gen * ((1.0 - pc) / denom)[:, None].astype(np.float32)
    out[:, VOCAB:] = res.results[0]["cpout"] * pc[:, None].astype(np.float32)
    return out


if __name__ == "__main__":
    # build-only smoke test
    nc = _get_nc()
    print("build OK:", nc)
